# revision 25
# baseline (speedup 1.0000x reference)
"""Trainium2 Bass kernel for nn_Attn_17738214933129.

Dense transformer attention block:
  Q/K/V projections from n_loc=2048 -> feat=512 (8 heads x 64),
  structural-bias softmax added to scaled QK^T scores, softmax, PV,
  output projection back to n_loc=2048.

Sharding: data-parallel over batch (16 -> 2 per core) across 8 NeuronCores,
weights replicated, no collectives.

Structure (per core, rows = 2*512 = 1024):
  - q/k pre-transposed+pre-tiled on host; QT[f,r], KT[f,r] via weight-stationary
    matmuls (contraction nl on partitions).
  - V computed NON-transposed directly (lhsT = vT chunks, rhs = WvT) into an
    augmented layout Vaug[k, h*65+d] with a ones column per head (h*65+64),
    produced by an extra contraction chunk in the weight itself.
  - Scores computed TRANSPOSED: ST[k,q] = KT_h^T-chunk vs QT_h rhs, so E lands
    in the [k, q] layout PV needs -- no P transposes at all.
  - E = exp(ST) * esmT where esm = softmax(masked str) is exponentiated once
    per (b,qt) and PE-transposed (small: 0.5M elems vs 4.2M for P).
  - PV: x_aug[q, h*65+d] = E_h^T-chunks @ Vaug_h; the ones column yields the
    softmax denominator per (q, head) for free; normalization is a cheap
    per-partition scale of x (8x smaller than scaling P).
  - xs[q,d] -> PE transpose (small) -> xsT -> output projection; out is
    written bf16 and cast to f32 on host.
"""

import sys

import numpy as np

try:
    import concourse.bass as bass  # noqa: F401
except Exception:  # pragma: no cover - path fallback
    sys.path.insert(0, "/opt/trn_rl_repo")

import ml_dtypes

import concourse.bacc as bacc
import concourse.tile as tile
from concourse import mybir
from concourse.bass_utils import run_bass_kernel_spmd

BF16 = mybir.dt.bfloat16
F32 = mybir.dt.float32
AF = mybir.ActivationFunctionType
ALU = mybir.AluOpType

B, S, NLOC = 16, 512, 2048
FEAT, H, DH = 512, 8, 64
NCORES = 8
BL = B // NCORES          # batch per core = 2
R = BL * S                # rows per core = 1024
KT_N = NLOC // 128        # 16 contraction tiles for projections
KV_N = KT_N + 1           # v has an extra all-(1/128) chunk for ones/bias
FT_N = FEAT // 128        # 4 feature tiles (head pairs)
QT_N = S // 128           # 4 query tiles per batch element
NL_N = NLOC // 512        # 4 output column chunks
VW = H * (DH + 1)         # 520: V augmented width (ones col per head)

_CACHE = {}


def _build(use_bias):
    nc = bacc.Bacc(
        "TRN2",
        target_bir_lowering=False,
        debug=False,
        enable_asserts=False,
        num_devices=NCORES,
    )
    KQ = KV_N if use_bias else KT_N  # q/k chunks (extra bias chunk if needed)

    # q/k/v pre-transposed and pre-tiled on host: [128, i*R + r] = x[r, i*128+p].
    d_q = nc.dram_tensor("q", [128, KQ * R], BF16, kind="ExternalInput").ap()
    d_k = nc.dram_tensor("k", [128, KQ * R], BF16, kind="ExternalInput").ap()
    d_v = nc.dram_tensor("v", [128, KV_N * R], BF16, kind="ExternalInput").ap()
    # masked str (where(mask==0,-1e9,str)) pre-tiled: [128, BL*QT_N*S].
    d_mstr = nc.dram_tensor("mstr", [128, BL * QT_N * S], F32, kind="ExternalInput").ap()
    # weights pre-tiled: wq/wk [128, KQ*512] with [p, i*512+f]=W.T[i*128+p, f];
    # wv augmented [128, KV_N*520]; wo [128, 4*2048] with [p, ft*2048+n].
    d_wq = nc.dram_tensor("wqT", [128, KQ * FEAT], BF16, kind="ExternalInput").ap()
    d_wk = nc.dram_tensor("wkT", [128, KQ * FEAT], BF16, kind="ExternalInput").ap()
    d_wv = nc.dram_tensor("wvT", [128, KV_N * VW], BF16, kind="ExternalInput").ap()
    d_wo = nc.dram_tensor("woT", [128, FT_N * NLOC], BF16, kind="ExternalInput").ap()
    d_bo = nc.dram_tensor("bor", [1, NLOC], BF16, kind="ExternalInput").ap()
    d_id = nc.dram_tensor("ident", [128, 128], BF16, kind="ExternalInput").ap()
    d_ones = nc.dram_tensor("onesr", [1, 512], BF16, kind="ExternalInput").ap()
    d_out = nc.dram_tensor("out", [R, NLOC], BF16, kind="ExternalOutput").ap()

    with tile.TileContext(nc) as tc:
        with (
            tc.tile_pool(name="consts", bufs=1) as cpool,
            tc.tile_pool(name="weights", bufs=1) as wpool,
            tc.tile_pool(name="persist", bufs=1) as ppool,
            tc.tile_pool(name="xtin", bufs=3) as spool,
            tc.tile_pool(name="mstr", bufs=4) as mpool,
            tc.tile_pool(name="smcol", bufs=1) as colpool,
            tc.tile_pool(name="e0", bufs=6) as epool,
            tc.tile_pool(name="ostage", bufs=2) as opool,
            tc.tile_pool(name="psum", bufs=8, space="PSUM") as psum,
        ):
            # ---- constants (vector queue; sync queue starts with wq0/q0) ----
            ident = cpool.tile([128, 128], BF16, tag="ident", name="ident")
            nc.scalar.dma_start(ident[:], d_id[:])
            ones = None
            bo_t = None
            if use_bias:
                ones = cpool.tile([1, 512], BF16, tag="ones", name="ones")
                nc.scalar.dma_start(ones[:], d_ones[:])
                bo_t = cpool.tile([1, NLOC], BF16, tag="bo", name="bo")
                nc.scalar.dma_start(bo_t[:], d_bo[:])

            # Persistent activations.
            QT = [ppool.tile([128, R], BF16, tag=f"QT{i}", name=f"QT{i}") for i in range(FT_N)]
            KTt = [ppool.tile([128, R], BF16, tag=f"KT{i}", name=f"KT{i}") for i in range(FT_N)]
            # Vaug[k-tile][p, h*65+d], col h*65+64 == 1.0 (+bv via weight chunk).
            Vaug = [ppool.tile([128, VW], BF16, tag=f"Va{i}", name=f"Va{i}") for i in range(R // 128)]
            esm = {}   # (b, qt) -> [128 q, 512 k] bf16 exp(softmax(mstr))
            esmT = {}  # (b, kt) -> [128 k, 512 q] bf16
            for b in range(BL):
                for j in range(QT_N):
                    esm[(b, j)] = ppool.tile([128, S], BF16, tag=f"esm{b}{j}", name=f"esm{b}{j}")
                    esmT[(b, j)] = ppool.tile([128, S], BF16, tag=f"esmT{b}{j}", name=f"esmT{b}{j}")
            # xs/xsT per b reuse the same tags (ring bufs=1 -> WAR-serialized, ok).
            usum = colpool.tile([128, BL * QT_N], F32, tag="usum", name="usum")
            ru = colpool.tile([128, BL * QT_N], F32, tag="ru", name="ru")

            # ---------- structural softmax (ACT-heavy; overlaps projections) --
            # DMAs + exps first (vector queue head), recips emitted later so
            # they don't block the bulk-weight DMA issues behind them.
            def emit_strsm_in():
                tiles = []
                for b in range(BL):
                    for qt in range(QT_N):
                        idx = b * QT_N + qt
                        mt = mpool.tile([128, S], F32, tag="mstr", name="mt")
                        nc.scalar.dma_start(
                            mt[:], d_mstr[:, idx * S : (idx + 1) * S]
                        )
                        u = mpool.tile([128, S], BF16, tag=f"u{idx}", name=f"u{idx}", bufs=1)
                        nc.scalar.activation(
                            u[:], mt[:], AF.Exp,
                            accum_out=usum[:, idx : idx + 1],
                        )
                        tiles.append((b, qt, idx, u))
                return tiles

            def emit_strsm_out(tiles):
                for b, qt, idx, u in tiles:
                    nc.vector.reciprocal(
                        ru[:, idx : idx + 1], usum[:, idx : idx + 1]
                    )
                    nc.scalar.activation(
                        esm[(b, qt)][:], u[:], AF.Exp,
                        scale=ru[:, idx : idx + 1],
                    )

            # ---------- projections ------------------------------------------
            def projection_T(dst, d_src, w, d_w, nch, dma_eng, xtag):
                """dst[f, r] tiles: transposed projection (Q and K)."""
                groups = {}
                for ft in range(FT_N):
                    for rc in range(R // 512):
                        ps = psum.tile([128, 512], F32, tag="ps", name="ps")
                        groups[(ft, rc)] = ps
                nchunks = KQ
                per = KT_N // nch
                for i in range(nchunks):
                    if i % per == 0 or i == KT_N:
                        c0 = i * FEAT
                        c1 = min(i + per, nchunks) * FEAT
                        dma_eng.dma_start(w[:, c0:c1], d_w[:, c0:c1])
                    xt = spool.tile([128, R], BF16, tag=xtag, name="xt_in")
                    dma_eng.dma_start(xt[:], d_src[:, i * R : (i + 1) * R])
                    for ft in range(FT_N):
                        for rc in range(R // 512):
                            nc.tensor.matmul(
                                groups[(ft, rc)][:],
                                lhsT=w[:, i * FEAT + ft * 128 : i * FEAT + (ft + 1) * 128],
                                rhs=xt[:, rc * 512 : (rc + 1) * 512],
                                start=(i == 0),
                                stop=(i == nchunks - 1),
                            )
                for ft in range(FT_N):
                    for rc in range(R // 512):
                        nc.vector.tensor_copy(
                            dst[ft][:, rc * 512 : (rc + 1) * 512],
                            groups[(ft, rc)][:],
                        )

            # Bulk inputs up-front on the vector queue (parallel DMA rings):
            # mstr for the structural softmax, v whole, wv, wo.
            strsm_tiles = emit_strsm_in()
            vbuf = wpool.tile([128, KV_N * R], BF16, tag="vbuf", name="vbuf")
            nc.gpsimd.dma_start(vbuf[:], d_v[:])
            wv = wpool.tile([128, KV_N * VW], BF16, tag="wv", name="wv")
            nc.gpsimd.dma_start(wv[:], d_wv[:])
            wo = wpool.tile([128, FT_N * NLOC], BF16, tag="wo", name="wo")
            nc.gpsimd.dma_start(wo[:], d_wo[:])
            emit_strsm_out(strsm_tiles)

            wq = wpool.tile([128, KQ * FEAT], BF16, tag="wq", name="wq")
            projection_T(QT, d_q, wq, d_wq, 8, nc.sync, "xTq")
            wk = wpool.tile([128, KQ * FEAT], BF16, tag="wk", name="wk")
            projection_T(KTt, d_k, wk, d_wk, 4, nc.gpsimd, "xTk")

            # esm -> esmT PE transposes (esm ready: ACT ran during projections).
            for b in range(BL):
                for kt in range(QT_N):
                    tp = psum.tile([128, 512], BF16, tag="ps", name="esmtp")
                    for qt in range(QT_N):
                        nc.tensor.matmul(
                            tp[:, qt * 128 : (qt + 1) * 128],
                            lhsT=esm[(b, qt)][:, kt * 128 : (kt + 1) * 128],
                            rhs=ident[:],
                            is_transpose=True,
                            start=(qt == 0),
                            stop=(qt == QT_N - 1),
                        )
                    nc.vector.tensor_copy(esmT[(b, kt)][:], tp[:])

            # ET per head-pair: [128 k, kt*1024 + hs*512 + q] bf16.
            ETp = [
                wpool.tile([128, QT_N * 1024], BF16, tag=("wq" if ht == 3 else f"ETp{ht}"), name=f"ETp{ht}")
                for ht in range(FT_N)
            ]
            ru8 = colpool.tile([128, BL * H], F32, tag="ru8", name="ru8")

            # ---- attention helpers ------------------------------------------
            def emit_score_unit(b, kt, ht, tt_eng):
                """One head-pair of transposed scores + exp + esm-mult."""
                for hs in range(2):
                    hb = hs * 64
                    sps = psum.tile([128, 512], F32, tag="ps", name="sps")
                    nc.tensor.matmul(
                        sps[:],
                        lhsT=KTt[ht][
                            hb : hb + 64,
                            b * S + kt * 128 : b * S + (kt + 1) * 128,
                        ],
                        rhs=QT[ht][hb : hb + 64, b * S : (b + 1) * S],
                        start=True,
                        stop=True,
                    )
                    e0 = epool.tile([128, S], BF16, tag="e0", name="e0")
                    nc.scalar.activation(e0[:], sps[:], AF.Exp)
                    tt_eng.tensor_tensor(
                        ETp[ht][:, kt * 1024 + hs * 512 : kt * 1024 + (hs + 1) * 512],
                        e0[:],
                        esmT[(b, kt)][:],
                        op=ALU.mult,
                    )

            def emit_pv_qt(b, qt, xs):
                """PV + denominators + normalize into xs (reads ETp)."""
                xa = [
                    psum.tile([128, 512], F32, tag="ps", name="xa")
                    for _ in range(2)
                ]
                for h in range(H):
                    ht, hs = h // 2, h % 2
                    pt = xa[h // 4]
                    c0 = (h % 4) * 65
                    for kt in range(QT_N):
                        nc.tensor.matmul(
                            pt[:, c0 : c0 + 65],
                            lhsT=ETp[ht][
                                :,
                                kt * 1024 + hs * 512 + qt * 128 : kt * 1024 + hs * 512 + (qt + 1) * 128,
                            ],
                            rhs=Vaug[b * QT_N + kt][:, h * 65 : (h + 1) * 65],
                            start=(kt == 0),
                            stop=(kt == QT_N - 1),
                        )
                for h in range(H):
                    pt = xa[h // 4]
                    c0 = (h % 4) * 65
                    col = b * H + h
                    nc.vector.reciprocal(
                        ru8[:, col : col + 1], pt[:, c0 + 64 : c0 + 65]
                    )
                    nc.vector.tensor_scalar(
                        xs[:, h * 64 : (h + 1) * 64],
                        pt[:, c0 : c0 + 64],
                        ru8[:, col : col + 1],
                        None,
                        op0=ALU.mult,
                    )

            def emit_out_qt(b, qt, xs, xsTq, feed, copy_alt):
                """Transpose + outproj for one (b, qt); safe to feed scores
                for the next b here (ETp no longer read)."""
                # xs[q, d] -> xsTq[d-block, q] (col block dt holds block-T).
                tp = psum.tile([128, 512], BF16, tag="ps", name="xstp")
                for dt in range(FT_N):
                    nc.tensor.matmul(
                        tp[:, dt * 128 : (dt + 1) * 128],
                        lhsT=xs[:, dt * 128 : (dt + 1) * 128],
                        rhs=ident[:],
                        is_transpose=True,
                        start=(dt == 0),
                        stop=(dt == FT_N - 1),
                    )
                nc.vector.tensor_copy(xsTq[:], tp[:])
                for nlc in range(NL_N):
                    if nlc % 2 == 0:
                        ot = opool.tile([128, 1024], BF16, tag="ot", name="ot")
                    for _ in range(2):
                        if feed:
                            emit_score_unit(*feed.pop(0))
                    ps = psum.tile([128, 512], F32, tag="ps", name="ops")
                    if use_bias:
                        nc.tensor.matmul(
                            ps[:],
                            lhsT=ones[0:1, 0:128],
                            rhs=bo_t[0:1, nlc * 512 : (nlc + 1) * 512],
                            start=True,
                            stop=False,
                        )
                    for dt in range(FT_N):
                        nc.tensor.matmul(
                            ps[:],
                            lhsT=xsTq[:, dt * 128 : (dt + 1) * 128],
                            rhs=wo[:, dt * NLOC + nlc * 512 : dt * NLOC + (nlc + 1) * 512],
                            start=(dt == 0 and not use_bias),
                            stop=(dt == FT_N - 1),
                        )
                    dst = ot[:, (nlc % 2) * 512 : (nlc % 2 + 1) * 512]
                    if copy_alt and nlc % 2 == 0:
                        nc.scalar.copy(dst, ps[:])
                    else:
                        nc.vector.tensor_copy(dst, ps[:])
                    if nlc % 2 == 1:
                        row0 = b * S + qt * 128
                        nc.sync.dma_start(
                            d_out[row0 : row0 + 128, (nlc - 1) * 512 : (nlc + 1) * 512],
                            ot[:],
                        )

            # ---- V projection interleaved with scores(b=0) ------------------
            # A few score units lead so PE has work if the vbuf DMA is late.
            s0 = [(0, kt, ht, nc.vector) for kt in range(QT_N) for ht in range(FT_N)]
            si = 0
            while si < 4:
                emit_score_unit(*s0[si])
                si += 1
            for rt in range(R // 128):
                for half in range(2):
                    vps = psum.tile([128, 260], F32, tag="ps", name="vps")
                    for i in range(KV_N):
                        nc.tensor.matmul(
                            vps[:],
                            lhsT=vbuf[:, i * R + rt * 128 : i * R + (rt + 1) * 128],
                            rhs=wv[:, i * VW + half * 260 : i * VW + (half + 1) * 260],
                            start=(i == 0),
                            stop=(i == KV_N - 1),
                        )
                    nc.vector.tensor_copy(
                        Vaug[rt][:, half * 260 : (half + 1) * 260], vps[:]
                    )
                    if si < len(s0) and (rt * 2 + half) % 4 != 3:
                        emit_score_unit(*s0[si])
                        si += 1
            while si < len(s0):
                emit_score_unit(*s0[si])
                si += 1

            # ---- attention tails: PV(b) fully first (ETp reads), then
            # transpose+outproj per qt (feeding next-b scores there) ----------
            s1 = [(1, kt, ht, nc.vector) for kt in range(QT_N) for ht in range(FT_N)]
            for b, feed, alt in ((0, s1, False), (1, s1, True)):
                xss, xsTqs = [], []
                for qt in range(QT_N):
                    xs = mpool.tile([128, 512], BF16, tag=f"u{qt}", name=f"xs{b}_{qt}", bufs=1)
                    xss.append(xs)
                    emit_pv_qt(b, qt, xs)
                for qt in range(QT_N):
                    xsTq = mpool.tile([128, 512], BF16, tag=f"u{4 + qt}", name=f"xsTq{b}_{qt}", bufs=1)
                    emit_out_qt(b, qt, xss[qt], xsTq, feed, copy_alt=alt)

    nc.compile()
    return nc


def _prep_inputs(q, k, v, str_mat, attn_mask, Wq, bq, Wk, bk, Wv, bv, Wo, bo):
    bf = ml_dtypes.bfloat16
    use_bias = bool(
        np.any(np.asarray(bq))
        or np.any(np.asarray(bk))
        or np.any(np.asarray(bv))
        or np.any(np.asarray(bo))
    )
    KQ = KV_N if use_bias else KT_N

    # 1/DH folded into Wq (and bq): DH = 64 = 2^6, exact in floating point.
    wqT = np.ascontiguousarray((Wq / np.float32(DH)).T).astype(np.float32)
    wkT = np.ascontiguousarray(Wk.T).astype(np.float32)
    woT = np.ascontiguousarray(Wo.T).astype(bf)

    # Pre-tile weights: [n*128, width] -> [128, n*width].
    def pretile(w):
        n = w.shape[0] // 128
        return np.ascontiguousarray(
            w.reshape(n, 128, w.shape[1]).transpose(1, 0, 2).reshape(128, -1)
        )

    if use_bias:
        wqT = np.concatenate([wqT, np.tile(bq[None, :] / np.float32(DH), (128, 1))], 0)
        wkT = np.concatenate([wkT, np.tile(bk[None, :], (128, 1))], 0)
    wqt = pretile(wqT.astype(bf))
    wkt = pretile(wkT.astype(bf))
    wot = pretile(woT)

    # Wv augmented+interleaved: chunk i, col h*65+d -> Wv.T[i*128+p, h*64+d];
    # col h*65+64 -> 0 for i<16; chunk 16 carries [bv | 1] (paired with the
    # all-(1/128) v chunk so sum_p (1/128)*x == x).
    wvT = Wv.T.astype(np.float32)  # [2048, 512]
    wva = np.zeros((KV_N * 128, VW), np.float32)
    for h in range(H):
        wva[:NLOC, h * 65 : h * 65 + 64] = wvT[:, h * 64 : (h + 1) * 64]
        wva[NLOC:, h * 65 : h * 65 + 64] = bv[None, h * 64 : (h + 1) * 64]
        wva[NLOC:, h * 65 + 64] = 1.0
    wvt = pretile(wva.astype(bf))

    bor = bo[None, :].astype(bf)
    ident = np.eye(128, dtype=bf)
    onesr = np.ones((1, 512), dtype=bf)

    q16 = np.asarray(q).astype(bf)
    k16 = np.asarray(k).astype(bf)
    v16 = np.asarray(v).astype(bf)
    onechunk = np.full((128, R), 1.0 / 128.0, dtype=bf)

    def pretile_T(x, aug):
        # [R, NLOC] -> [128, n*R] with [p, i*R+r] = x[r, i*128+p]
        t = np.ascontiguousarray(
            x.reshape(R, KT_N, 128).transpose(2, 1, 0).reshape(128, KT_N * R)
        )
        if aug:
            t = np.concatenate([t, onechunk], axis=1)
        return np.ascontiguousarray(t)

    strf = np.asarray(str_mat, dtype=np.float32)
    maskf = np.asarray(attn_mask) != 0
    mstr = np.where(maskf, strf, np.float32(-1e9))

    in_maps = []
    for c in range(NCORES):
        sl = slice(c * BL, (c + 1) * BL)
        # [BL, S, S] -> [128, BL*QT_N*S] with [p, (b*4+qt)*S+col].
        mstrt = np.ascontiguousarray(
            mstr[sl].reshape(BL * QT_N, 128, S).transpose(1, 0, 2).reshape(128, -1)
        )
        m = {
            "q": pretile_T(q16[sl].reshape(R, NLOC), use_bias),
            "k": pretile_T(k16[sl].reshape(R, NLOC), use_bias),
            "v": pretile_T(v16[sl].reshape(R, NLOC), True),
            "mstr": mstrt,
            "wqT": wqt,
            "wkT": wkt,
            "wvT": wvt,
            "woT": wot,
            "bor": bor,
            "ident": ident,
            "onesr": onesr,
        }
        in_maps.append(m)
    return in_maps, use_bias


def kernel(q, k, v, str_mat, attn_mask, Wq, bq, Wk, bk, Wv, bv, Wo, bo):
    in_maps, use_bias = _prep_inputs(
        q, k, v, str_mat, attn_mask, Wq, bq, Wk, bk, Wv, bv, Wo, bo
    )
    key = ("nc", use_bias)
    if key not in _CACHE:
        _CACHE[key] = _build(use_bias)
    nc = _CACHE[key]
    res = run_bass_kernel_spmd(nc, in_maps, core_ids=list(range(NCORES)))
    out = np.empty((B, S, NLOC), dtype=np.float32)
    for c in range(NCORES):
        out[c * BL : (c + 1) * BL] = (
            res.results[c]["out"].astype(np.float32).reshape(BL, S, NLOC)
        )
    return out


# revision 26
# speedup vs baseline: 1.0555x; 1.0555x over previous
"""Trainium2 Bass kernel for nn_Attn_17738214933129.

Dense transformer attention block:
  Q/K/V projections from n_loc=2048 -> feat=512 (8 heads x 64),
  structural-bias softmax added to scaled QK^T scores, softmax, PV,
  output projection back to n_loc=2048.

Sharding: data-parallel over batch (16 -> 2 per core) across 8 NeuronCores,
weights replicated, no collectives.

Structure (per core, rows = 2*512 = 1024):
  - q/k pre-transposed+pre-tiled on host; QT[f,r], KT[f,r] via weight-stationary
    matmuls (contraction nl on partitions).
  - V computed NON-transposed directly (lhsT = vT chunks, rhs = WvT) into an
    augmented layout Vaug[k, h*65+d] with a ones column per head (h*65+64),
    produced by an extra contraction chunk in the weight itself.
  - Scores computed TRANSPOSED: ST[k,q] = KT_h^T-chunk vs QT_h rhs, so E lands
    in the [k, q] layout PV needs -- no P transposes at all.
  - E = exp(ST) * esmT where esm = softmax(masked str) is exponentiated once
    per (b,qt) and PE-transposed (small: 0.5M elems vs 4.2M for P).
  - PV: x_aug[q, h*65+d] = E_h^T-chunks @ Vaug_h; the ones column yields the
    softmax denominator per (q, head) for free; normalization is a cheap
    per-partition scale of x (8x smaller than scaling P).
  - xs[q,d] -> PE transpose (small) -> xsT -> output projection; out is
    written bf16 and cast to f32 on host.
"""

import sys

import numpy as np

try:
    import concourse.bass as bass  # noqa: F401
except Exception:  # pragma: no cover - path fallback
    sys.path.insert(0, "/opt/trn_rl_repo")

import ml_dtypes

import concourse.bacc as bacc
import concourse.tile as tile
from concourse import mybir
from concourse.bass_utils import run_bass_kernel_spmd

BF16 = mybir.dt.bfloat16
F32 = mybir.dt.float32
AF = mybir.ActivationFunctionType
ALU = mybir.AluOpType

B, S, NLOC = 16, 512, 2048
FEAT, H, DH = 512, 8, 64
NCORES = 8
BL = B // NCORES          # batch per core = 2
R = BL * S                # rows per core = 1024
KT_N = NLOC // 128        # 16 contraction tiles for projections
KV_N = KT_N + 1           # v has an extra all-(1/128) chunk for ones/bias
FT_N = FEAT // 128        # 4 feature tiles (head pairs)
QT_N = S // 128           # 4 query tiles per batch element
NL_N = NLOC // 512        # 4 output column chunks
VW = H * (DH + 1)         # 520: V augmented width (ones col per head)

_CACHE = {}


def _build(use_bias):
    nc = bacc.Bacc(
        "TRN2",
        target_bir_lowering=False,
        debug=False,
        enable_asserts=False,
        num_devices=NCORES,
    )
    KQ = KV_N if use_bias else KT_N  # q/k chunks (extra bias chunk if needed)

    # q/k/v pre-transposed and pre-tiled on host: [128, i*R + r] = x[r, i*128+p].
    d_q = nc.dram_tensor("q", [128, KQ * R], BF16, kind="ExternalInput").ap()
    d_k = nc.dram_tensor("k", [128, KQ * R], BF16, kind="ExternalInput").ap()
    d_v = nc.dram_tensor("v", [128, KV_N * R], BF16, kind="ExternalInput").ap()
    # masked str (where(mask==0,-1e9,str)) pre-tiled: [128, BL*QT_N*S].
    d_mstr = nc.dram_tensor("mstr", [128, BL * QT_N * S], F32, kind="ExternalInput").ap()
    # weights pre-tiled: wq/wk [128, KQ*512] with [p, i*512+f]=W.T[i*128+p, f];
    # wv augmented [128, KV_N*520]; wo [128, 4*2048] with [p, ft*2048+n].
    d_wq = nc.dram_tensor("wqT", [128, KQ * FEAT], BF16, kind="ExternalInput").ap()
    d_wk = nc.dram_tensor("wkT", [128, KQ * FEAT], BF16, kind="ExternalInput").ap()
    d_wv = nc.dram_tensor("wvT", [128, KV_N * VW], BF16, kind="ExternalInput").ap()
    d_wo = nc.dram_tensor("woT", [128, FT_N * NLOC], BF16, kind="ExternalInput").ap()
    d_bo = nc.dram_tensor("bor", [1, NLOC], BF16, kind="ExternalInput").ap()
    d_id = nc.dram_tensor("ident", [128, 128], BF16, kind="ExternalInput").ap()
    d_ones = nc.dram_tensor("onesr", [1, 512], BF16, kind="ExternalInput").ap()
    d_out = nc.dram_tensor("out", [R, NLOC], BF16, kind="ExternalOutput").ap()

    with tile.TileContext(nc) as tc:
        with (
            tc.tile_pool(name="consts", bufs=1) as cpool,
            tc.tile_pool(name="weights", bufs=1) as wpool,
            tc.tile_pool(name="persist", bufs=1) as ppool,
            tc.tile_pool(name="xtin", bufs=3) as spool,
            tc.tile_pool(name="mstr", bufs=4) as mpool,
            tc.tile_pool(name="smcol", bufs=1) as colpool,
            tc.tile_pool(name="e0", bufs=6) as epool,
            tc.tile_pool(name="ostage", bufs=2) as opool,
            tc.tile_pool(name="psum", bufs=8, space="PSUM") as psum,
        ):
            # ---- constants (vector queue; sync queue starts with wq0/q0) ----
            ident = cpool.tile([128, 128], BF16, tag="ident", name="ident")
            nc.scalar.dma_start(ident[:], d_id[:])
            ones = None
            bo_t = None
            if use_bias:
                ones = cpool.tile([1, 512], BF16, tag="ones", name="ones")
                nc.scalar.dma_start(ones[:], d_ones[:])
                bo_t = cpool.tile([1, NLOC], BF16, tag="bo", name="bo")
                nc.scalar.dma_start(bo_t[:], d_bo[:])

            # Persistent activations.
            QT = [ppool.tile([128, R], BF16, tag=f"QT{i}", name=f"QT{i}") for i in range(FT_N)]
            KTt = [ppool.tile([128, R], BF16, tag=f"KT{i}", name=f"KT{i}") for i in range(FT_N)]
            # Vaug[k-tile][p, h*65+d], col h*65+64 == 1.0 (+bv via weight chunk).
            Vaug = [ppool.tile([128, VW], BF16, tag=f"Va{i}", name=f"Va{i}") for i in range(R // 128)]
            esm = {}   # (b, qt) -> [128 q, 512 k] bf16 exp(softmax(mstr))
            esmT = {}  # (b, kt) -> [128 k, 512 q] bf16
            for b in range(BL):
                for j in range(QT_N):
                    esm[(b, j)] = ppool.tile([128, S], BF16, tag=f"esm{b}{j}", name=f"esm{b}{j}")
                    esmT[(b, j)] = ppool.tile([128, S], BF16, tag=f"esmT{b}{j}", name=f"esmT{b}{j}")
            # xs/xsT per b reuse the same tags (ring bufs=1 -> WAR-serialized, ok).
            usum = colpool.tile([128, BL * QT_N], F32, tag="usum", name="usum")
            ru = colpool.tile([128, BL * QT_N], F32, tag="ru", name="ru")

            # ---------- structural softmax (ACT-heavy; overlaps projections) --
            # DMAs + exps first (vector queue head), recips emitted later so
            # they don't block the bulk-weight DMA issues behind them.
            def emit_strsm_in():
                tiles = []
                for b in range(BL):
                    for qt in range(QT_N):
                        idx = b * QT_N + qt
                        mt = mpool.tile([128, S], F32, tag="mstr", name="mt")
                        nc.scalar.dma_start(
                            mt[:], d_mstr[:, idx * S : (idx + 1) * S]
                        )
                        u = mpool.tile([128, S], BF16, tag=f"u{idx}", name=f"u{idx}", bufs=1)
                        nc.scalar.activation(
                            u[:], mt[:], AF.Exp,
                            accum_out=usum[:, idx : idx + 1],
                        )
                        tiles.append((b, qt, idx, u))
                return tiles

            def emit_strsm_out(tiles):
                for b, qt, idx, u in tiles:
                    nc.vector.reciprocal(
                        ru[:, idx : idx + 1], usum[:, idx : idx + 1]
                    )
                    nc.scalar.activation(
                        esm[(b, qt)][:], u[:], AF.Exp,
                        scale=ru[:, idx : idx + 1],
                    )

            # ---------- projections ------------------------------------------
            def projection_T(dst, d_src, w, d_w, nch, dma_eng, xtag):
                """dst[f, r] tiles: transposed projection (Q and K)."""
                groups = {}
                for ft in range(FT_N):
                    for rc in range(R // 512):
                        ps = psum.tile([128, 512], F32, tag="ps", name="ps")
                        groups[(ft, rc)] = ps
                nchunks = KQ
                per = KT_N // nch
                for i in range(nchunks):
                    if i % per == 0 or i == KT_N:
                        c0 = i * FEAT
                        c1 = min(i + per, nchunks) * FEAT
                        dma_eng.dma_start(w[:, c0:c1], d_w[:, c0:c1])
                    xt = spool.tile([128, R], BF16, tag=xtag, name="xt_in")
                    dma_eng.dma_start(xt[:], d_src[:, i * R : (i + 1) * R])
                    for ft in range(FT_N):
                        for rc in range(R // 512):
                            nc.tensor.matmul(
                                groups[(ft, rc)][:],
                                lhsT=w[:, i * FEAT + ft * 128 : i * FEAT + (ft + 1) * 128],
                                rhs=xt[:, rc * 512 : (rc + 1) * 512],
                                start=(i == 0),
                                stop=(i == nchunks - 1),
                            )
                for ft in range(FT_N):
                    for rc in range(R // 512):
                        nc.vector.tensor_copy(
                            dst[ft][:, rc * 512 : (rc + 1) * 512],
                            groups[(ft, rc)][:],
                        )

            # Bulk inputs up-front on the vector queue (parallel DMA rings):
            # mstr for the structural softmax, v whole, wv, wo.
            strsm_tiles = emit_strsm_in()
            vbuf = wpool.tile([128, KV_N * R], BF16, tag="vbuf", name="vbuf")
            nc.scalar.dma_start(vbuf[:], d_v[:])
            wv = wpool.tile([128, KV_N * VW], BF16, tag="wv", name="wv")
            nc.scalar.dma_start(wv[:], d_wv[:])
            wo = wpool.tile([128, FT_N * NLOC], BF16, tag="wo", name="wo")
            nc.scalar.dma_start(wo[:], d_wo[:])
            emit_strsm_out(strsm_tiles)

            wq = wpool.tile([128, KQ * FEAT], BF16, tag="wq", name="wq")
            projection_T(QT, d_q, wq, d_wq, 8, nc.sync, "xTq")
            wk = wpool.tile([128, KQ * FEAT], BF16, tag="wk", name="wk")
            projection_T(KTt, d_k, wk, d_wk, 4, nc.scalar, "xTk")

            # esm -> esmT PE transposes (esm ready: ACT ran during projections).
            for b in range(BL):
                for kt in range(QT_N):
                    tp = psum.tile([128, 512], BF16, tag="ps", name="esmtp")
                    for qt in range(QT_N):
                        nc.tensor.matmul(
                            tp[:, qt * 128 : (qt + 1) * 128],
                            lhsT=esm[(b, qt)][:, kt * 128 : (kt + 1) * 128],
                            rhs=ident[:],
                            is_transpose=True,
                            start=(qt == 0),
                            stop=(qt == QT_N - 1),
                        )
                    nc.vector.tensor_copy(esmT[(b, kt)][:], tp[:])

            # ET per head-pair: [128 k, kt*1024 + hs*512 + q] bf16.
            ETp = [
                wpool.tile([128, QT_N * 1024], BF16, tag=("wq" if ht == 3 else f"ETp{ht}"), name=f"ETp{ht}")
                for ht in range(FT_N)
            ]
            ru8 = colpool.tile([128, BL * H], F32, tag="ru8", name="ru8")

            # ---- attention helpers ------------------------------------------
            def emit_score_unit(b, kt, ht, tt_eng):
                """One head-pair of transposed scores + exp + esm-mult."""
                for hs in range(2):
                    hb = hs * 64
                    sps = psum.tile([128, 512], F32, tag="ps", name="sps")
                    nc.tensor.matmul(
                        sps[:],
                        lhsT=KTt[ht][
                            hb : hb + 64,
                            b * S + kt * 128 : b * S + (kt + 1) * 128,
                        ],
                        rhs=QT[ht][hb : hb + 64, b * S : (b + 1) * S],
                        start=True,
                        stop=True,
                    )
                    e0 = epool.tile([128, S], BF16, tag="e0", name="e0")
                    nc.scalar.activation(e0[:], sps[:], AF.Exp)
                    tt_eng.tensor_tensor(
                        ETp[ht][:, kt * 1024 + hs * 512 : kt * 1024 + (hs + 1) * 512],
                        e0[:],
                        esmT[(b, kt)][:],
                        op=ALU.mult,
                    )

            def emit_pv_qt(b, qt, xs):
                """PV + denominators + normalize into xs (reads ETp)."""
                xa = [
                    psum.tile([128, 512], F32, tag="ps", name="xa")
                    for _ in range(2)
                ]
                for h in range(H):
                    ht, hs = h // 2, h % 2
                    pt = xa[h // 4]
                    c0 = (h % 4) * 65
                    for kt in range(QT_N):
                        nc.tensor.matmul(
                            pt[:, c0 : c0 + 65],
                            lhsT=ETp[ht][
                                :,
                                kt * 1024 + hs * 512 + qt * 128 : kt * 1024 + hs * 512 + (qt + 1) * 128,
                            ],
                            rhs=Vaug[b * QT_N + kt][:, h * 65 : (h + 1) * 65],
                            start=(kt == 0),
                            stop=(kt == QT_N - 1),
                        )
                for h in range(H):
                    pt = xa[h // 4]
                    c0 = (h % 4) * 65
                    col = b * H + h
                    nc.vector.reciprocal(
                        ru8[:, col : col + 1], pt[:, c0 + 64 : c0 + 65]
                    )
                    nc.vector.tensor_scalar(
                        xs[:, h * 64 : (h + 1) * 64],
                        pt[:, c0 : c0 + 64],
                        ru8[:, col : col + 1],
                        None,
                        op0=ALU.mult,
                    )

            def emit_out_qt(b, qt, xs, xsTq, feed, copy_alt):
                """Transpose + outproj for one (b, qt); safe to feed scores
                for the next b here (ETp no longer read)."""
                # xs[q, d] -> xsTq[d-block, q] (col block dt holds block-T).
                tp = psum.tile([128, 512], BF16, tag="ps", name="xstp")
                for dt in range(FT_N):
                    nc.tensor.matmul(
                        tp[:, dt * 128 : (dt + 1) * 128],
                        lhsT=xs[:, dt * 128 : (dt + 1) * 128],
                        rhs=ident[:],
                        is_transpose=True,
                        start=(dt == 0),
                        stop=(dt == FT_N - 1),
                    )
                nc.vector.tensor_copy(xsTq[:], tp[:])
                for nlc in range(NL_N):
                    if nlc % 2 == 0:
                        ot = opool.tile([128, 1024], BF16, tag="ot", name="ot")
                    for _ in range(2):
                        if feed:
                            emit_score_unit(*feed.pop(0))
                    ps = psum.tile([128, 512], F32, tag="ps", name="ops")
                    if use_bias:
                        nc.tensor.matmul(
                            ps[:],
                            lhsT=ones[0:1, 0:128],
                            rhs=bo_t[0:1, nlc * 512 : (nlc + 1) * 512],
                            start=True,
                            stop=False,
                        )
                    for dt in range(FT_N):
                        nc.tensor.matmul(
                            ps[:],
                            lhsT=xsTq[:, dt * 128 : (dt + 1) * 128],
                            rhs=wo[:, dt * NLOC + nlc * 512 : dt * NLOC + (nlc + 1) * 512],
                            start=(dt == 0 and not use_bias),
                            stop=(dt == FT_N - 1),
                        )
                    dst = ot[:, (nlc % 2) * 512 : (nlc % 2 + 1) * 512]
                    if copy_alt and nlc % 2 == 0:
                        nc.scalar.copy(dst, ps[:])
                    else:
                        nc.vector.tensor_copy(dst, ps[:])
                    if nlc % 2 == 1:
                        row0 = b * S + qt * 128
                        nc.sync.dma_start(
                            d_out[row0 : row0 + 128, (nlc - 1) * 512 : (nlc + 1) * 512],
                            ot[:],
                        )

            # ---- V projection interleaved with scores(b=0) ------------------
            # A few score units lead so PE has work if the vbuf DMA is late.
            s0 = [(0, kt, ht, nc.vector) for kt in range(QT_N) for ht in range(FT_N)]
            si = 0
            while si < 4:
                emit_score_unit(*s0[si])
                si += 1
            for rt in range(R // 128):
                for half in range(2):
                    vps = psum.tile([128, 260], F32, tag="ps", name="vps")
                    for i in range(KV_N):
                        nc.tensor.matmul(
                            vps[:],
                            lhsT=vbuf[:, i * R + rt * 128 : i * R + (rt + 1) * 128],
                            rhs=wv[:, i * VW + half * 260 : i * VW + (half + 1) * 260],
                            start=(i == 0),
                            stop=(i == KV_N - 1),
                        )
                    nc.vector.tensor_copy(
                        Vaug[rt][:, half * 260 : (half + 1) * 260], vps[:]
                    )
                    if si < len(s0) and (rt * 2 + half) % 4 != 3:
                        emit_score_unit(*s0[si])
                        si += 1
            while si < len(s0):
                emit_score_unit(*s0[si])
                si += 1

            # ---- attention tails: PV(b) fully first (ETp reads), then
            # transpose+outproj per qt (feeding next-b scores there) ----------
            s1 = [(1, kt, ht, nc.vector) for kt in range(QT_N) for ht in range(FT_N)]
            for b, feed, alt in ((0, s1, False), (1, s1, True)):
                xss, xsTqs = [], []
                for qt in range(QT_N):
                    xs = mpool.tile([128, 512], BF16, tag=f"u{qt}", name=f"xs{b}_{qt}", bufs=1)
                    xss.append(xs)
                    emit_pv_qt(b, qt, xs)
                for qt in range(QT_N):
                    xsTq = mpool.tile([128, 512], BF16, tag=f"u{4 + qt}", name=f"xsTq{b}_{qt}", bufs=1)
                    emit_out_qt(b, qt, xss[qt], xsTq, feed, copy_alt=alt)

    nc.compile()
    return nc


def _prep_inputs(q, k, v, str_mat, attn_mask, Wq, bq, Wk, bk, Wv, bv, Wo, bo):
    bf = ml_dtypes.bfloat16
    use_bias = bool(
        np.any(np.asarray(bq))
        or np.any(np.asarray(bk))
        or np.any(np.asarray(bv))
        or np.any(np.asarray(bo))
    )
    KQ = KV_N if use_bias else KT_N

    # 1/DH folded into Wq (and bq): DH = 64 = 2^6, exact in floating point.
    wqT = np.ascontiguousarray((Wq / np.float32(DH)).T).astype(np.float32)
    wkT = np.ascontiguousarray(Wk.T).astype(np.float32)
    woT = np.ascontiguousarray(Wo.T).astype(bf)

    # Pre-tile weights: [n*128, width] -> [128, n*width].
    def pretile(w):
        n = w.shape[0] // 128
        return np.ascontiguousarray(
            w.reshape(n, 128, w.shape[1]).transpose(1, 0, 2).reshape(128, -1)
        )

    if use_bias:
        wqT = np.concatenate([wqT, np.tile(bq[None, :] / np.float32(DH), (128, 1))], 0)
        wkT = np.concatenate([wkT, np.tile(bk[None, :], (128, 1))], 0)
    wqt = pretile(wqT.astype(bf))
    wkt = pretile(wkT.astype(bf))
    wot = pretile(woT)

    # Wv augmented+interleaved: chunk i, col h*65+d -> Wv.T[i*128+p, h*64+d];
    # col h*65+64 -> 0 for i<16; chunk 16 carries [bv | 1] (paired with the
    # all-(1/128) v chunk so sum_p (1/128)*x == x).
    wvT = Wv.T.astype(np.float32)  # [2048, 512]
    wva = np.zeros((KV_N * 128, VW), np.float32)
    for h in range(H):
        wva[:NLOC, h * 65 : h * 65 + 64] = wvT[:, h * 64 : (h + 1) * 64]
        wva[NLOC:, h * 65 : h * 65 + 64] = bv[None, h * 64 : (h + 1) * 64]
        wva[NLOC:, h * 65 + 64] = 1.0
    wvt = pretile(wva.astype(bf))

    bor = bo[None, :].astype(bf)
    ident = np.eye(128, dtype=bf)
    onesr = np.ones((1, 512), dtype=bf)

    q16 = np.asarray(q).astype(bf)
    k16 = np.asarray(k).astype(bf)
    v16 = np.asarray(v).astype(bf)
    onechunk = np.full((128, R), 1.0 / 128.0, dtype=bf)

    def pretile_T(x, aug):
        # [R, NLOC] -> [128, n*R] with [p, i*R+r] = x[r, i*128+p]
        t = np.ascontiguousarray(
            x.reshape(R, KT_N, 128).transpose(2, 1, 0).reshape(128, KT_N * R)
        )
        if aug:
            t = np.concatenate([t, onechunk], axis=1)
        return np.ascontiguousarray(t)

    strf = np.asarray(str_mat, dtype=np.float32)
    maskf = np.asarray(attn_mask) != 0
    mstr = np.where(maskf, strf, np.float32(-1e9))

    in_maps = []
    for c in range(NCORES):
        sl = slice(c * BL, (c + 1) * BL)
        # [BL, S, S] -> [128, BL*QT_N*S] with [p, (b*4+qt)*S+col].
        mstrt = np.ascontiguousarray(
            mstr[sl].reshape(BL * QT_N, 128, S).transpose(1, 0, 2).reshape(128, -1)
        )
        m = {
            "q": pretile_T(q16[sl].reshape(R, NLOC), use_bias),
            "k": pretile_T(k16[sl].reshape(R, NLOC), use_bias),
            "v": pretile_T(v16[sl].reshape(R, NLOC), True),
            "mstr": mstrt,
            "wqT": wqt,
            "wkT": wkt,
            "wvT": wvt,
            "woT": wot,
            "bor": bor,
            "ident": ident,
            "onesr": onesr,
        }
        in_maps.append(m)
    return in_maps, use_bias


def kernel(q, k, v, str_mat, attn_mask, Wq, bq, Wk, bk, Wv, bv, Wo, bo):
    in_maps, use_bias = _prep_inputs(
        q, k, v, str_mat, attn_mask, Wq, bq, Wk, bk, Wv, bv, Wo, bo
    )
    key = ("nc", use_bias)
    if key not in _CACHE:
        _CACHE[key] = _build(use_bias)
    nc = _CACHE[key]
    res = run_bass_kernel_spmd(nc, in_maps, core_ids=list(range(NCORES)))
    out = np.empty((B, S, NLOC), dtype=np.float32)
    for c in range(NCORES):
        out[c * BL : (c + 1) * BL] = (
            res.results[c]["out"].astype(np.float32).reshape(BL, S, NLOC)
        )
    return out


# revision 27
# speedup vs baseline: 1.0674x; 1.0113x over previous
"""Trainium2 Bass kernel for nn_Attn_17738214933129.

Dense transformer attention block:
  Q/K/V projections from n_loc=2048 -> feat=512 (8 heads x 64),
  structural-bias softmax added to scaled QK^T scores, softmax, PV,
  output projection back to n_loc=2048.

Sharding: data-parallel over batch (16 -> 2 per core) across 8 NeuronCores,
weights replicated, no collectives.

Structure (per core, rows = 2*512 = 1024):
  - q/k pre-transposed+pre-tiled on host; QT[f,r], KT[f,r] via weight-stationary
    matmuls (contraction nl on partitions).
  - V computed NON-transposed directly (lhsT = vT chunks, rhs = WvT) into an
    augmented layout Vaug[k, h*65+d] with a ones column per head (h*65+64),
    produced by an extra contraction chunk in the weight itself.
  - Scores computed TRANSPOSED: ST[k,q] = KT_h-chunk^T @ QT_h, so E lands in
    the [k, q] layout PV needs -- no P transposes at all.
  - E = exp(ST) * esmT where esm = softmax(masked str) is exponentiated once
    per (b,qt) and PE-transposed (small: 0.5M elems vs 4.2M for P).
  - PV: x_aug[q, h*65+d] = E_h^T-chunks @ Vaug_h; the ones column yields the
    softmax denominator per (q, head) for free; normalization is a cheap
    per-partition scale of x (8x smaller than scaling P).
  - xs[q,d] -> PE transpose (small) -> xsTq -> output projection; out is
    written bf16 and cast to f32 on host.
  - All PSUM tiles are [128,1024] two-bank tiles: score head-pairs share one
    tile so exp runs on [128,1024] (halves ACT instruction count), outproj
    nlc-pairs share one tile (halves PSUM->SBUF copy count).
  - DMA rings: SP carries wq+q stream and out tiles; ACT carries ident, mstr,
    wk+k stream with vbuf/wv/wo issues staggered between k chunks so the bulk
    transfers land during K-proj instead of starving the q stream.
"""

import sys

import numpy as np

try:
    import concourse.bass as bass  # noqa: F401
except Exception:  # pragma: no cover - path fallback
    sys.path.insert(0, "/opt/trn_rl_repo")

import ml_dtypes

import concourse.bacc as bacc
import concourse.tile as tile
from concourse import mybir
from concourse.bass_utils import run_bass_kernel_spmd

BF16 = mybir.dt.bfloat16
F32 = mybir.dt.float32
AF = mybir.ActivationFunctionType
ALU = mybir.AluOpType

B, S, NLOC = 16, 512, 2048
FEAT, H, DH = 512, 8, 64
NCORES = 8
BL = B // NCORES          # batch per core = 2
R = BL * S                # rows per core = 1024
KT_N = NLOC // 128        # 16 contraction tiles for projections
KV_N = KT_N + 1           # v has an extra all-(1/128) chunk for ones/bias
FT_N = FEAT // 128        # 4 feature tiles (head pairs)
QT_N = S // 128           # 4 query tiles per batch element
NL_N = NLOC // 512        # 4 output column chunks
VW = H * (DH + 1)         # 520: V augmented width (ones col per head)

_CACHE = {}


def _build(use_bias):
    nc = bacc.Bacc(
        "TRN2",
        target_bir_lowering=False,
        debug=False,
        enable_asserts=False,
        num_devices=NCORES,
    )
    KQ = KV_N if use_bias else KT_N  # q/k chunks (extra bias chunk if needed)

    # q/k/v pre-transposed and pre-tiled on host: [128, i*R + r] = x[r, i*128+p].
    d_q = nc.dram_tensor("q", [128, KQ * R], BF16, kind="ExternalInput").ap()
    d_k = nc.dram_tensor("k", [128, KQ * R], BF16, kind="ExternalInput").ap()
    d_v = nc.dram_tensor("v", [128, KV_N * R], BF16, kind="ExternalInput").ap()
    # masked str (where(mask==0,-1e9,str)) pre-tiled: [128, BL*QT_N*S].
    d_mstr = nc.dram_tensor("mstr", [128, BL * QT_N * S], F32, kind="ExternalInput").ap()
    # weights pre-tiled: wq/wk [128, KQ*512] with [p, i*512+f]=W.T[i*128+p, f];
    # wv augmented [128, KV_N*520]; wo [128, 4*2048] with [p, ft*2048+n].
    d_wq = nc.dram_tensor("wqT", [128, KQ * FEAT], BF16, kind="ExternalInput").ap()
    d_wk = nc.dram_tensor("wkT", [128, KQ * FEAT], BF16, kind="ExternalInput").ap()
    d_wv = nc.dram_tensor("wvT", [128, KV_N * VW], BF16, kind="ExternalInput").ap()
    d_wo = nc.dram_tensor("woT", [128, FT_N * NLOC], BF16, kind="ExternalInput").ap()
    d_bo = nc.dram_tensor("bor", [1, NLOC], BF16, kind="ExternalInput").ap()
    d_id = nc.dram_tensor("ident", [128, 128], BF16, kind="ExternalInput").ap()
    d_ones = nc.dram_tensor("onesr", [1, 512], BF16, kind="ExternalInput").ap()
    d_out = nc.dram_tensor("out", [R, NLOC], BF16, kind="ExternalOutput").ap()

    with tile.TileContext(nc) as tc:
        with (
            tc.tile_pool(name="consts", bufs=1) as cpool,
            tc.tile_pool(name="weights", bufs=1) as wpool,
            tc.tile_pool(name="persist", bufs=1) as ppool,
            tc.tile_pool(name="xtin", bufs=3) as spool,
            tc.tile_pool(name="mstr", bufs=3) as mpool,
            tc.tile_pool(name="smcol", bufs=1) as colpool,
            tc.tile_pool(name="e0", bufs=3) as epool,
            tc.tile_pool(name="ostage", bufs=2) as opool,
            tc.tile_pool(name="psum", bufs=4, space="PSUM") as psum,
        ):
            def big_f32():
                return psum.tile([128, 1024], F32, tag="big", name="bps")

            def big_bf16():
                return psum.tile([128, 1024], BF16, tag="big", name="bps16")

            # ---- constants (ACT ring; SP ring starts with wq0/q0) -----------
            ident = cpool.tile([128, 128], BF16, tag="ident", name="ident")
            nc.scalar.dma_start(ident[:], d_id[:])
            ones = None
            bo_t = None
            if use_bias:
                ones = cpool.tile([1, 512], BF16, tag="ones", name="ones")
                nc.scalar.dma_start(ones[:], d_ones[:])
                bo_t = cpool.tile([1, NLOC], BF16, tag="bo", name="bo")
                nc.scalar.dma_start(bo_t[:], d_bo[:])

            # Persistent activations.
            QT = [ppool.tile([128, R], BF16, tag=f"QT{i}", name=f"QT{i}") for i in range(FT_N)]
            KTt = [ppool.tile([128, R], BF16, tag=f"KT{i}", name=f"KT{i}") for i in range(FT_N)]
            # Vaug[k-tile][p, h*65+d], col h*65+64 == 1.0 (+bv via weight chunk).
            Vaug = [ppool.tile([128, VW], BF16, tag=f"Va{i}", name=f"Va{i}") for i in range(R // 128)]
            esm = {}   # (b, qt) -> [128 q, 512 k] bf16 exp(softmax(mstr))
            esmT = {}  # (b, kt) -> [128 k, 512 q] bf16
            for b in range(BL):
                for j in range(QT_N):
                    esm[(b, j)] = ppool.tile([128, S], BF16, tag=f"esm{b}{j}", name=f"esm{b}{j}")
                    esmT[(b, j)] = ppool.tile([128, S], BF16, tag=f"esmT{b}{j}", name=f"esmT{b}{j}")
            usum = colpool.tile([128, BL * QT_N], F32, tag="usum", name="usum")
            ru = colpool.tile([128, BL * QT_N], F32, tag="ru", name="ru")

            # ---------- structural softmax (ACT; overlaps projections) -------
            def emit_strsm_in():
                tiles = []
                for b in range(BL):
                    for qt in range(QT_N):
                        idx = b * QT_N + qt
                        mt = mpool.tile([128, S], F32, tag="mstr", name="mt")
                        nc.scalar.dma_start(
                            mt[:], d_mstr[:, idx * S : (idx + 1) * S]
                        )
                        u = mpool.tile([128, S], BF16, tag=f"u{idx}", name=f"u{idx}", bufs=1)
                        nc.scalar.activation(
                            u[:], mt[:], AF.Exp,
                            accum_out=usum[:, idx : idx + 1],
                        )
                        tiles.append((b, qt, idx, u))
                return tiles

            def emit_strsm_out(tiles):
                for b, qt, idx, u in tiles:
                    nc.vector.reciprocal(
                        ru[:, idx : idx + 1], usum[:, idx : idx + 1]
                    )
                    nc.scalar.activation(
                        esm[(b, qt)][:], u[:], AF.Exp,
                        scale=ru[:, idx : idx + 1],
                    )

            # ---------- projections ------------------------------------------
            def projection_T(dst, d_src, w, d_w, nch, dma_eng, xtag, extra=None):
                """dst[f, r] tiles: transposed projection (Q and K).

                extra: optional list of (dst_ap, src_ap) bulk DMAs issued one
                per few chunks so their transfers stagger across this stream.
                """
                groups = [big_f32() for _ in range(FT_N)]
                nchunks = KQ
                per = KT_N // nch
                for i in range(nchunks):
                    if i % per == 0 or i == KT_N:
                        c0 = i * FEAT
                        c1 = min(i + per, nchunks) * FEAT
                        dma_eng.dma_start(w[:, c0:c1], d_w[:, c0:c1])
                    xt = spool.tile([128, R], BF16, tag=xtag, name="xt_in")
                    dma_eng.dma_start(xt[:], d_src[:, i * R : (i + 1) * R])
                    if extra and i % 3 == 1:
                        dst_ap, src_ap = extra.pop(0)
                        dma_eng.dma_start(dst_ap, src_ap)
                    for ft in range(FT_N):
                        for rc in range(R // 512):
                            nc.tensor.matmul(
                                groups[ft][:, rc * 512 : (rc + 1) * 512],
                                lhsT=w[:, i * FEAT + ft * 128 : i * FEAT + (ft + 1) * 128],
                                rhs=xt[:, rc * 512 : (rc + 1) * 512],
                                start=(i == 0),
                                stop=(i == nchunks - 1),
                            )
                while extra:
                    dst_ap, src_ap = extra.pop(0)
                    dma_eng.dma_start(dst_ap, src_ap)
                for ft in range(FT_N):
                    nc.vector.tensor_copy(dst[ft][:], groups[ft][:])

            strsm_tiles = emit_strsm_in()
            emit_strsm_out(strsm_tiles)

            vbuf = wpool.tile([128, KV_N * R], BF16, tag="vbuf", name="vbuf")
            wv = wpool.tile([128, KV_N * VW], BF16, tag="wv", name="wv")
            wo = wpool.tile([128, FT_N * NLOC], BF16, tag="wo", name="wo")
            VB4 = KV_N * R // 4  # 4352
            bulk = [
                (vbuf[:, j * VB4 : (j + 1) * VB4], d_v[:, j * VB4 : (j + 1) * VB4])
                for j in range(4)
            ] + [(wv[:], d_wv[:]), (wo[:], d_wo[:])]

            wq = wpool.tile([128, KQ * FEAT], BF16, tag="wq", name="wq")
            projection_T(QT, d_q, wq, d_wq, 8, nc.sync, "xTq")
            wk = wpool.tile([128, KQ * FEAT], BF16, tag="wk", name="wk")
            projection_T(KTt, d_k, wk, d_wk, 4, nc.scalar, "xTk", extra=bulk)

            # esm -> esmT PE transposes (esm ready: ACT ran during proj).
            for b in range(BL):
                for kp in range(2):
                    tp = big_bf16()
                    for sub in range(2):
                        kt = kp * 2 + sub
                        for qt in range(QT_N):
                            nc.tensor.matmul(
                                tp[:, sub * 512 + qt * 128 : sub * 512 + (qt + 1) * 128],
                                lhsT=esm[(b, qt)][:, kt * 128 : (kt + 1) * 128],
                                rhs=ident[:],
                                is_transpose=True,
                                start=(qt == 0),
                                stop=(qt == QT_N - 1),
                            )
                    for sub in range(2):
                        nc.vector.tensor_copy(
                            esmT[(b, kp * 2 + sub)][:],
                            tp[:, sub * 512 : (sub + 1) * 512],
                        )

            # ET per head-pair: [128 k, kt*1024 + hs*512 + q] bf16.
            ETp = [
                wpool.tile([128, QT_N * 1024], BF16, tag=("wq" if ht == 3 else f"ETp{ht}"), name=f"ETp{ht}")
                for ht in range(FT_N)
            ]
            ru8 = colpool.tile([128, BL * H], F32, tag="ru8", name="ru8")

            # ---- attention helpers ------------------------------------------
            def emit_score_unit(b, kt, ht):
                """One head-pair of transposed scores + one wide exp + esm-mult."""
                sps = big_f32()
                for hs in range(2):
                    hb = hs * 64
                    nc.tensor.matmul(
                        sps[:, hs * 512 : (hs + 1) * 512],
                        lhsT=KTt[ht][
                            hb : hb + 64,
                            b * S + kt * 128 : b * S + (kt + 1) * 128,
                        ],
                        rhs=QT[ht][hb : hb + 64, b * S : (b + 1) * S],
                        start=True,
                        stop=True,
                    )
                e0 = epool.tile([128, 1024], BF16, tag="e0", name="e0")
                nc.scalar.activation(e0[:], sps[:], AF.Exp)
                for hs in range(2):
                    nc.vector.tensor_tensor(
                        ETp[ht][:, kt * 1024 + hs * 512 : kt * 1024 + (hs + 1) * 512],
                        e0[:, hs * 512 : (hs + 1) * 512],
                        esmT[(b, kt)][:],
                        op=ALU.mult,
                    )

            def emit_pv_qt(b, qt, xs):
                """PV + denominators + normalize into xs (reads ETp)."""
                xa = big_f32()
                for h in range(H):
                    ht, hs = h // 2, h % 2
                    c0 = (h // 4) * 512 + (h % 4) * 65
                    for kt in range(QT_N):
                        nc.tensor.matmul(
                            xa[:, c0 : c0 + 65],
                            lhsT=ETp[ht][
                                :,
                                kt * 1024 + hs * 512 + qt * 128 : kt * 1024 + hs * 512 + (qt + 1) * 128,
                            ],
                            rhs=Vaug[b * QT_N + kt][:, h * 65 : (h + 1) * 65],
                            start=(kt == 0),
                            stop=(kt == QT_N - 1),
                        )
                for h in range(H):
                    c0 = (h // 4) * 512 + (h % 4) * 65
                    col = b * H + h
                    nc.vector.reciprocal(
                        ru8[:, col : col + 1], xa[:, c0 + 64 : c0 + 65]
                    )
                    nc.vector.tensor_scalar(
                        xs[:, h * 64 : (h + 1) * 64],
                        xa[:, c0 : c0 + 64],
                        ru8[:, col : col + 1],
                        None,
                        op0=ALU.mult,
                    )

            def emit_out_qp(b, qp, xs_pair, xsTq2, feed, alt):
                """Transpose + outproj for a qt pair; next-b scores fed here
                (ETp no longer read by this b)."""
                tp = big_bf16()
                for sub in range(2):
                    for dt in range(FT_N):
                        nc.tensor.matmul(
                            tp[:, sub * 512 + dt * 128 : sub * 512 + (dt + 1) * 128],
                            lhsT=xs_pair[sub][:, dt * 128 : (dt + 1) * 128],
                            rhs=ident[:],
                            is_transpose=True,
                            start=(dt == 0),
                            stop=(dt == FT_N - 1),
                        )
                nc.vector.tensor_copy(xsTq2[:], tp[:])
                for sub in range(2):
                    qt = qp * 2 + sub
                    for np_ in range(2):  # nlc pairs
                        for _ in range(2):
                            if feed:
                                emit_score_unit(*feed.pop(0))
                        ps = big_f32()
                        ot = opool.tile([128, 1024], BF16, tag="ot", name="ot")
                        for half in range(2):
                            nlc = np_ * 2 + half
                            o = ps[:, half * 512 : (half + 1) * 512]
                            if use_bias:
                                nc.tensor.matmul(
                                    o,
                                    lhsT=ones[0:1, 0:128],
                                    rhs=bo_t[0:1, nlc * 512 : (nlc + 1) * 512],
                                    start=True,
                                    stop=False,
                                )
                            for dt in range(FT_N):
                                nc.tensor.matmul(
                                    o,
                                    lhsT=xsTq2[:, sub * 512 + dt * 128 : sub * 512 + (dt + 1) * 128],
                                    rhs=wo[:, dt * NLOC + nlc * 512 : dt * NLOC + (nlc + 1) * 512],
                                    start=(dt == 0 and not use_bias),
                                    stop=(dt == FT_N - 1),
                                )
                        if alt and np_ % 2 == 1:
                            nc.scalar.copy(ot[:], ps[:])
                        else:
                            nc.vector.tensor_copy(ot[:], ps[:])
                        row0 = b * S + qt * 128
                        nc.sync.dma_start(
                            d_out[row0 : row0 + 128, np_ * 1024 : (np_ + 1) * 1024],
                            ot[:],
                        )

            # ---- V projection interleaved with scores(b=0) ------------------
            s0 = [(0, kt, ht) for kt in range(QT_N) for ht in range(FT_N)]
            si = 0
            while si < 4:
                emit_score_unit(*s0[si])
                si += 1
            for rt in range(R // 128):
                vps = big_f32()
                for half in range(2):
                    for i in range(KV_N):
                        nc.tensor.matmul(
                            vps[:, half * 512 : half * 512 + 260],
                            lhsT=vbuf[:, i * R + rt * 128 : i * R + (rt + 1) * 128],
                            rhs=wv[:, i * VW + half * 260 : i * VW + (half + 1) * 260],
                            start=(i == 0),
                            stop=(i == KV_N - 1),
                        )
                    if si < len(s0) and (rt * 2 + half) % 3 != 2:
                        emit_score_unit(*s0[si])
                        si += 1
                for half in range(2):
                    nc.vector.tensor_copy(
                        Vaug[rt][:, half * 260 : (half + 1) * 260],
                        vps[:, half * 512 : half * 512 + 260],
                    )
            while si < len(s0):
                emit_score_unit(*s0[si])
                si += 1

            # ---- attention tails: PV(b) fully first (ETp reads), then
            # transpose+outproj per qt-pair (feeding next-b scores there) -----
            s1 = [(1, kt, ht) for kt in range(QT_N) for ht in range(FT_N)]
            for b, feed in ((0, s1), (1, [])):
                xss = []
                for qt in range(QT_N):
                    xs = mpool.tile([128, 512], BF16, tag=f"u{qt}", name=f"xs{b}_{qt}", bufs=1)
                    xss.append(xs)
                    emit_pv_qt(b, qt, xs)
                for qp in range(2):
                    xsTq2 = mpool.tile(
                        [128, 1024], BF16, tag=f"u{4 + 2 * qp}", name=f"xsTq{b}_{qp}", bufs=1
                    )
                    emit_out_qp(b, qp, xss[2 * qp : 2 * qp + 2], xsTq2, feed, alt=True)

    nc.compile()
    return nc


def _prep_inputs(q, k, v, str_mat, attn_mask, Wq, bq, Wk, bk, Wv, bv, Wo, bo):
    bf = ml_dtypes.bfloat16
    use_bias = bool(
        np.any(np.asarray(bq))
        or np.any(np.asarray(bk))
        or np.any(np.asarray(bv))
        or np.any(np.asarray(bo))
    )
    KQ = KV_N if use_bias else KT_N

    # 1/DH folded into Wq (and bq): DH = 64 = 2^6, exact in floating point.
    wqT = np.ascontiguousarray((Wq / np.float32(DH)).T).astype(np.float32)
    wkT = np.ascontiguousarray(Wk.T).astype(np.float32)
    woT = np.ascontiguousarray(Wo.T).astype(bf)

    # Pre-tile weights: [n*128, width] -> [128, n*width].
    def pretile(w):
        n = w.shape[0] // 128
        return np.ascontiguousarray(
            w.reshape(n, 128, w.shape[1]).transpose(1, 0, 2).reshape(128, -1)
        )

    if use_bias:
        wqT = np.concatenate([wqT, np.tile(bq[None, :] / np.float32(DH), (128, 1))], 0)
        wkT = np.concatenate([wkT, np.tile(bk[None, :], (128, 1))], 0)
    wqt = pretile(wqT.astype(bf))
    wkt = pretile(wkT.astype(bf))
    wot = pretile(woT)

    # Wv augmented+interleaved: chunk i, col h*65+d -> Wv.T[i*128+p, h*64+d];
    # col h*65+64 -> 0 for i<16; chunk 16 carries [bv | 1] (paired with the
    # all-(1/128) v chunk so sum_p (1/128)*x == x).
    wvT = Wv.T.astype(np.float32)  # [2048, 512]
    wva = np.zeros((KV_N * 128, VW), np.float32)
    for h in range(H):
        wva[:NLOC, h * 65 : h * 65 + 64] = wvT[:, h * 64 : (h + 1) * 64]
        wva[NLOC:, h * 65 : h * 65 + 64] = bv[None, h * 64 : (h + 1) * 64]
        wva[NLOC:, h * 65 + 64] = 1.0
    wvt = pretile(wva.astype(bf))

    bor = bo[None, :].astype(bf)
    ident = np.eye(128, dtype=bf)
    onesr = np.ones((1, 512), dtype=bf)

    q16 = np.asarray(q).astype(bf)
    k16 = np.asarray(k).astype(bf)
    v16 = np.asarray(v).astype(bf)
    onechunk = np.full((128, R), 1.0 / 128.0, dtype=bf)

    def pretile_T(x, aug):
        # [R, NLOC] -> [128, n*R] with [p, i*R+r] = x[r, i*128+p]
        t = np.ascontiguousarray(
            x.reshape(R, KT_N, 128).transpose(2, 1, 0).reshape(128, KT_N * R)
        )
        if aug:
            t = np.concatenate([t, onechunk], axis=1)
        return np.ascontiguousarray(t)

    strf = np.asarray(str_mat, dtype=np.float32)
    maskf = np.asarray(attn_mask) != 0
    mstr = np.where(maskf, strf, np.float32(-1e9))

    in_maps = []
    for c in range(NCORES):
        sl = slice(c * BL, (c + 1) * BL)
        # [BL, S, S] -> [128, BL*QT_N*S] with [p, (b*4+qt)*S+col].
        mstrt = np.ascontiguousarray(
            mstr[sl].reshape(BL * QT_N, 128, S).transpose(1, 0, 2).reshape(128, -1)
        )
        m = {
            "q": pretile_T(q16[sl].reshape(R, NLOC), use_bias),
            "k": pretile_T(k16[sl].reshape(R, NLOC), use_bias),
            "v": pretile_T(v16[sl].reshape(R, NLOC), True),
            "mstr": mstrt,
            "wqT": wqt,
            "wkT": wkt,
            "wvT": wvt,
            "woT": wot,
            "bor": bor,
            "ident": ident,
            "onesr": onesr,
        }
        in_maps.append(m)
    return in_maps, use_bias


def kernel(q, k, v, str_mat, attn_mask, Wq, bq, Wk, bk, Wv, bv, Wo, bo):
    in_maps, use_bias = _prep_inputs(
        q, k, v, str_mat, attn_mask, Wq, bq, Wk, bk, Wv, bv, Wo, bo
    )
    key = ("nc", use_bias)
    if key not in _CACHE:
        _CACHE[key] = _build(use_bias)
    nc = _CACHE[key]
    res = run_bass_kernel_spmd(nc, in_maps, core_ids=list(range(NCORES)))
    out = np.empty((B, S, NLOC), dtype=np.float32)
    for c in range(NCORES):
        out[c * BL : (c + 1) * BL] = (
            res.results[c]["out"].astype(np.float32).reshape(BL, S, NLOC)
        )
    return out


# revision 29
# speedup vs baseline: 1.1270x; 1.0559x over previous
"""Trainium2 Bass kernel for nn_Attn_17738214933129.

Dense transformer attention block:
  Q/K/V projections from n_loc=2048 -> feat=512 (8 heads x 64),
  structural-bias softmax added to scaled QK^T scores, softmax, PV,
  output projection back to n_loc=2048.

Sharding: data-parallel over batch (16 -> 2 per core) across 8 NeuronCores,
weights replicated, no collectives.

Structure (per core, rows = 2*512 = 1024):
  - q/k pre-transposed+pre-tiled on host; QT[f,r], KT[f,r] via weight-stationary
    matmuls (contraction nl on partitions).
  - V computed NON-transposed directly (lhsT = vT chunks, rhs = WvT) into an
    augmented layout Vaug[k, h*65+d] with a ones column per head (h*65+64),
    produced by an extra contraction chunk in the weight itself.
  - Scores computed TRANSPOSED: ST[k,q] = KT_h-chunk^T @ QT_h, so E lands in
    the [k, q] layout PV needs -- no P transposes at all.
  - E = exp(ST) * esmT where esm = softmax(masked str) is exponentiated once
    per (b,qt) and PE-transposed (small: 0.5M elems vs 4.2M for P).
  - PV: x_aug[q, h*65+d] = E_h^T-chunks @ Vaug_h; the ones column yields the
    softmax denominator per (q, head) for free; normalization is a cheap
    per-partition scale of x (8x smaller than scaling P).
  - xs[q,d] -> PE transpose (small) -> xsTq -> output projection; out is
    written bf16 and cast to f32 on host.
  - All PSUM tiles are [128,1024] two-bank tiles: score head-pairs share one
    tile so exp runs on [128,1024] (halves ACT instruction count), outproj
    nlc-pairs share one tile (halves PSUM->SBUF copy count).
  - DMA rings: SP carries wq+q stream and out tiles; ACT carries ident, mstr,
    wk+k stream with vbuf/wv/wo issues staggered between k chunks so the bulk
    transfers land during K-proj instead of starving the q stream.
"""

import sys

import numpy as np

try:
    import concourse.bass as bass  # noqa: F401
except Exception:  # pragma: no cover - path fallback
    sys.path.insert(0, "/opt/trn_rl_repo")

import ml_dtypes

import concourse.bacc as bacc
import concourse.tile as tile
from concourse import mybir
from concourse.bass_utils import run_bass_kernel_spmd

BF16 = mybir.dt.bfloat16
F32 = mybir.dt.float32
AF = mybir.ActivationFunctionType
ALU = mybir.AluOpType

B, S, NLOC = 16, 512, 2048
FEAT, H, DH = 512, 8, 64
NCORES = 8
BL = B // NCORES          # batch per core = 2
R = BL * S                # rows per core = 1024
KT_N = NLOC // 128        # 16 contraction tiles for projections
KV_N = KT_N + 1           # v has an extra all-(1/128) chunk for ones/bias
FT_N = FEAT // 128        # 4 feature tiles (head pairs)
QT_N = S // 128           # 4 query tiles per batch element
NL_N = NLOC // 512        # 4 output column chunks
VW = H * (DH + 1)         # 520: V augmented width (ones col per head)

_CACHE = {}


def _build(use_bias):
    nc = bacc.Bacc(
        "TRN2",
        target_bir_lowering=False,
        debug=False,
        enable_asserts=False,
        num_devices=NCORES,
    )
    KQ = KV_N if use_bias else KT_N  # q/k chunks (extra bias chunk if needed)

    # q/k/v pre-transposed and pre-tiled on host: [128, i*R + r] = x[r, i*128+p].
    d_q = nc.dram_tensor("q", [128, KQ * R], BF16, kind="ExternalInput").ap()
    d_k = nc.dram_tensor("k", [128, KQ * R], BF16, kind="ExternalInput").ap()
    d_v = nc.dram_tensor("v", [128, KV_N * R], BF16, kind="ExternalInput").ap()
    # masked str (where(mask==0,-1e9,str)) pre-tiled: [128, BL*QT_N*S].
    d_mstr = nc.dram_tensor("mstr", [128, BL * QT_N * S], F32, kind="ExternalInput").ap()
    # weights pre-tiled: wq/wk [128, KQ*512] with [p, i*512+f]=W.T[i*128+p, f];
    # wv augmented [128, KV_N*520]; wo [128, 4*2048] with [p, ft*2048+n].
    d_wq = nc.dram_tensor("wqT", [128, KQ * FEAT], BF16, kind="ExternalInput").ap()
    d_wk = nc.dram_tensor("wkT", [128, KQ * FEAT], BF16, kind="ExternalInput").ap()
    d_wv = nc.dram_tensor("wvT", [128, KV_N * VW], BF16, kind="ExternalInput").ap()
    d_wo = nc.dram_tensor("woT", [128, FT_N * NLOC], BF16, kind="ExternalInput").ap()
    d_bo = nc.dram_tensor("bor", [1, NLOC], BF16, kind="ExternalInput").ap()
    d_id = nc.dram_tensor("ident", [128, 128], BF16, kind="ExternalInput").ap()
    d_ones = nc.dram_tensor("onesr", [1, 512], BF16, kind="ExternalInput").ap()
    d_out = nc.dram_tensor("out", [R, NLOC], BF16, kind="ExternalOutput").ap()

    with tile.TileContext(nc) as tc:
        with (
            tc.tile_pool(name="consts", bufs=1) as cpool,
            tc.tile_pool(name="weights", bufs=1) as wpool,
            tc.tile_pool(name="persist", bufs=1) as ppool,
            tc.tile_pool(name="xtin", bufs=4) as spool,
            tc.tile_pool(name="mstr", bufs=3) as mpool,
            tc.tile_pool(name="smcol", bufs=1) as colpool,
            tc.tile_pool(name="e0", bufs=2) as epool,
            tc.tile_pool(name="ostage", bufs=2) as opool,
            tc.tile_pool(name="psum", bufs=4, space="PSUM") as psum,
        ):
            def big_f32():
                return psum.tile([128, 1024], F32, tag="big", name="bps")

            def big_bf16():
                return psum.tile([128, 1024], BF16, tag="big", name="bps16")

            # ---- constants (ACT ring; SP ring starts with wq0/q0) -----------
            ident = cpool.tile([128, 128], BF16, tag="ident", name="ident")
            nc.scalar.dma_start(ident[:], d_id[:])
            ones = None
            bo_t = None
            if use_bias:
                ones = cpool.tile([1, 512], BF16, tag="ones", name="ones")
                nc.scalar.dma_start(ones[:], d_ones[:])
                bo_t = cpool.tile([1, NLOC], BF16, tag="bo", name="bo")
                nc.scalar.dma_start(bo_t[:], d_bo[:])

            # Persistent activations.
            QT = [ppool.tile([128, R], BF16, tag=f"QT{i}", name=f"QT{i}") for i in range(FT_N)]
            KTt = [ppool.tile([128, R], BF16, tag=f"KT{i}", name=f"KT{i}") for i in range(FT_N)]
            # Vaug[k-tile][p, h*65+d], col h*65+64 == 1.0 (+bv via weight chunk).
            Vaug = [ppool.tile([128, VW], BF16, tag=f"Va{i}", name=f"Va{i}") for i in range(R // 128)]
            esm = {}   # (b, qt) -> [128 q, 512 k] bf16 exp(softmax(mstr))
            esmT = {}  # (b, kt) -> [128 k, 512 q] bf16
            for b in range(BL):
                for j in range(QT_N):
                    esm[(b, j)] = ppool.tile([128, S], BF16, tag=f"esm{b}{j}", name=f"esm{b}{j}")
                    esmT[(b, j)] = ppool.tile([128, S], BF16, tag=f"esmT{b}{j}", name=f"esmT{b}{j}")
            usum = colpool.tile([128, BL * QT_N], F32, tag="usum", name="usum")
            ru = colpool.tile([128, BL * QT_N], F32, tag="ru", name="ru")

            # ---------- structural softmax (ACT; overlaps projections) -------
            def emit_strsm_in():
                tiles = []
                for b in range(BL):
                    for qt in range(QT_N):
                        idx = b * QT_N + qt
                        mt = mpool.tile([128, S], F32, tag="mstr", name="mt")
                        nc.scalar.dma_start(
                            mt[:], d_mstr[:, idx * S : (idx + 1) * S]
                        )
                        u = mpool.tile([128, S], BF16, tag=f"u{idx}", name=f"u{idx}", bufs=1)
                        nc.scalar.activation(
                            u[:], mt[:], AF.Exp,
                            accum_out=usum[:, idx : idx + 1],
                        )
                        tiles.append((b, qt, idx, u))
                return tiles

            def emit_strsm_out(tiles):
                for b, qt, idx, u in tiles:
                    nc.vector.reciprocal(
                        ru[:, idx : idx + 1], usum[:, idx : idx + 1]
                    )
                    nc.scalar.activation(
                        esm[(b, qt)][:], u[:], AF.Exp,
                        scale=ru[:, idx : idx + 1],
                    )

            # ---------- projections ------------------------------------------
            def projection_T(dst, d_src, w, d_w, nch, dma_eng, xtag, extra=None):
                """dst[f, r] tiles: transposed projection (Q and K).

                extra: optional list of (dst_ap, src_ap) bulk DMAs issued one
                per few chunks so their transfers stagger across this stream.
                """
                groups = [big_f32() for _ in range(FT_N)]
                nchunks = KQ
                per = KT_N // nch
                for i in range(nchunks):
                    if i % per == 0 or i == KT_N:
                        c0 = i * FEAT
                        c1 = min(i + per, nchunks) * FEAT
                        dma_eng.dma_start(w[:, c0:c1], d_w[:, c0:c1])
                    xt = spool.tile([128, R], BF16, tag=xtag, name="xt_in")
                    dma_eng.dma_start(xt[:], d_src[:, i * R : (i + 1) * R])
                    if extra and i % 3 == 1:
                        dst_ap, src_ap = extra.pop(0)
                        dma_eng.dma_start(dst_ap, src_ap)
                    for ft in range(FT_N):
                        for rc in range(R // 512):
                            nc.tensor.matmul(
                                groups[ft][:, rc * 512 : (rc + 1) * 512],
                                lhsT=w[:, i * FEAT + ft * 128 : i * FEAT + (ft + 1) * 128],
                                rhs=xt[:, rc * 512 : (rc + 1) * 512],
                                start=(i == 0),
                                stop=(i == nchunks - 1),
                            )
                while extra:
                    dst_ap, src_ap = extra.pop(0)
                    dma_eng.dma_start(dst_ap, src_ap)
                for ft in range(FT_N):
                    nc.vector.tensor_copy(dst[ft][:], groups[ft][:])

            vbuf = wpool.tile([128, KV_N * R], BF16, tag="vbuf", name="vbuf")
            wv = wpool.tile([128, KV_N * VW], BF16, tag="wv", name="wv")
            wo = wpool.tile([128, FT_N * NLOC], BF16, tag="wo", name="wo")

            # mstr + structural exps early on the ACT ring (small transfers).
            strsm_tiles = emit_strsm_in()
            emit_strsm_out(strsm_tiles)
            wq = wpool.tile([128, KQ * FEAT], BF16, tag="wq", name="wq")
            projection_T(QT, d_q, wq, d_wq, 8, nc.sync, "xTq")
            # SP ring is idle once the q stream drains: issue the bulk there
            # so the transfers land during K-proj (whose stream is on ACT).
            VB4 = KV_N * R // 4  # 4352
            for j in range(4):
                nc.sync.dma_start(
                    vbuf[:, j * VB4 : (j + 1) * VB4],
                    d_v[:, j * VB4 : (j + 1) * VB4],
                )
            nc.sync.dma_start(wv[:], d_wv[:])
            nc.sync.dma_start(wo[:], d_wo[:])
            wk = wpool.tile([128, KQ * FEAT], BF16, tag="wk", name="wk")
            projection_T(KTt, d_k, wk, d_wk, 4, nc.scalar, "xTk")

            # esm -> esmT PE transposes (esm ready: ACT ran during proj).
            for b in range(BL):
                for kp in range(2):
                    tp = big_bf16()
                    for sub in range(2):
                        kt = kp * 2 + sub
                        for qt in range(QT_N):
                            nc.tensor.matmul(
                                tp[:, sub * 512 + qt * 128 : sub * 512 + (qt + 1) * 128],
                                lhsT=esm[(b, qt)][:, kt * 128 : (kt + 1) * 128],
                                rhs=ident[:],
                                is_transpose=True,
                                start=(qt == 0),
                                stop=(qt == QT_N - 1),
                            )
                    for sub in range(2):
                        nc.vector.tensor_copy(
                            esmT[(b, kp * 2 + sub)][:],
                            tp[:, sub * 512 : (sub + 1) * 512],
                        )

            # ET per head-pair: [128 k, kt*1024 + hs*512 + q] bf16.
            ETp = [
                wpool.tile([128, QT_N * 1024], BF16, tag=("wq" if ht == 3 else f"ETp{ht}"), name=f"ETp{ht}")
                for ht in range(FT_N)
            ]
            ru8 = colpool.tile([128, BL * H], F32, tag="ru8", name="ru8")

            # ---- attention helpers ------------------------------------------
            def emit_score_unit(b, kt, ht):
                """One head-pair of transposed scores + one wide exp + esm-mult."""
                sps = big_f32()
                for hs in range(2):
                    hb = hs * 64
                    nc.tensor.matmul(
                        sps[:, hs * 512 : (hs + 1) * 512],
                        lhsT=KTt[ht][
                            hb : hb + 64,
                            b * S + kt * 128 : b * S + (kt + 1) * 128,
                        ],
                        rhs=QT[ht][hb : hb + 64, b * S : (b + 1) * S],
                        start=True,
                        stop=True,
                    )
                e0 = epool.tile([128, 1024], BF16, tag="e0", name="e0")
                nc.scalar.activation(e0[:], sps[:], AF.Exp)
                for hs in range(2):
                    nc.vector.tensor_tensor(
                        ETp[ht][:, kt * 1024 + hs * 512 : kt * 1024 + (hs + 1) * 512],
                        e0[:, hs * 512 : (hs + 1) * 512],
                        esmT[(b, kt)][:],
                        op=ALU.mult,
                    )

            def emit_pv_qt(b, qt, xs):
                """PV + denominators + normalize into xs (reads ETp)."""
                xa = big_f32()
                for h in range(H):
                    ht, hs = h // 2, h % 2
                    c0 = (h // 4) * 512 + (h % 4) * 65
                    for kt in range(QT_N):
                        nc.tensor.matmul(
                            xa[:, c0 : c0 + 65],
                            lhsT=ETp[ht][
                                :,
                                kt * 1024 + hs * 512 + qt * 128 : kt * 1024 + hs * 512 + (qt + 1) * 128,
                            ],
                            rhs=Vaug[b * QT_N + kt][:, h * 65 : (h + 1) * 65],
                            start=(kt == 0),
                            stop=(kt == QT_N - 1),
                        )
                for h in range(H):
                    c0 = (h // 4) * 512 + (h % 4) * 65
                    col = b * H + h
                    nc.vector.reciprocal(
                        ru8[:, col : col + 1], xa[:, c0 + 64 : c0 + 65]
                    )
                    nc.vector.tensor_scalar(
                        xs[:, h * 64 : (h + 1) * 64],
                        xa[:, c0 : c0 + 64],
                        ru8[:, col : col + 1],
                        None,
                        op0=ALU.mult,
                    )

            def emit_out_qp(b, qp, xs_pair, xsTq2, feed, alt):
                """Transpose + outproj for a qt pair; next-b scores fed here
                (ETp no longer read by this b)."""
                tp = big_bf16()
                for sub in range(2):
                    for dt in range(FT_N):
                        nc.tensor.matmul(
                            tp[:, sub * 512 + dt * 128 : sub * 512 + (dt + 1) * 128],
                            lhsT=xs_pair[sub][:, dt * 128 : (dt + 1) * 128],
                            rhs=ident[:],
                            is_transpose=True,
                            start=(dt == 0),
                            stop=(dt == FT_N - 1),
                        )
                nc.vector.tensor_copy(xsTq2[:], tp[:])
                for sub in range(2):
                    qt = qp * 2 + sub
                    for np_ in range(2):  # nlc pairs
                        for _ in range(2):
                            if feed:
                                emit_score_unit(*feed.pop(0))
                        ps = big_f32()
                        ot = opool.tile([128, 1024], BF16, tag="ot", name="ot")
                        for half in range(2):
                            nlc = np_ * 2 + half
                            o = ps[:, half * 512 : (half + 1) * 512]
                            if use_bias:
                                nc.tensor.matmul(
                                    o,
                                    lhsT=ones[0:1, 0:128],
                                    rhs=bo_t[0:1, nlc * 512 : (nlc + 1) * 512],
                                    start=True,
                                    stop=False,
                                )
                            for dt in range(FT_N):
                                nc.tensor.matmul(
                                    o,
                                    lhsT=xsTq2[:, sub * 512 + dt * 128 : sub * 512 + (dt + 1) * 128],
                                    rhs=wo[:, dt * NLOC + nlc * 512 : dt * NLOC + (nlc + 1) * 512],
                                    start=(dt == 0 and not use_bias),
                                    stop=(dt == FT_N - 1),
                                )
                        if alt and np_ % 2 == 1:
                            nc.scalar.copy(ot[:], ps[:])
                        else:
                            nc.vector.tensor_copy(ot[:], ps[:])
                        row0 = b * S + qt * 128
                        nc.sync.dma_start(
                            d_out[row0 : row0 + 128, np_ * 1024 : (np_ + 1) * 1024],
                            ot[:],
                        )

            # ---- V projection interleaved with scores(b=0) ------------------
            s0 = [(0, kt, ht) for kt in range(QT_N) for ht in range(FT_N)]
            si = 0
            while si < 4:
                emit_score_unit(*s0[si])
                si += 1
            for rt in range(R // 128):
                vps = big_f32()
                for half in range(2):
                    for i in range(KV_N):
                        nc.tensor.matmul(
                            vps[:, half * 512 : half * 512 + 260],
                            lhsT=vbuf[:, i * R + rt * 128 : i * R + (rt + 1) * 128],
                            rhs=wv[:, i * VW + half * 260 : i * VW + (half + 1) * 260],
                            start=(i == 0),
                            stop=(i == KV_N - 1),
                        )
                    if si < len(s0) and (rt * 2 + half) % 3 != 2:
                        emit_score_unit(*s0[si])
                        si += 1
                for half in range(2):
                    nc.vector.tensor_copy(
                        Vaug[rt][:, half * 260 : (half + 1) * 260],
                        vps[:, half * 512 : half * 512 + 260],
                    )
            while si < len(s0):
                emit_score_unit(*s0[si])
                si += 1

            # ---- attention tails: PV(b) fully first (ETp reads), then
            # transpose+outproj per qt-pair (feeding next-b scores there) -----
            s1 = [(1, kt, ht) for kt in range(QT_N) for ht in range(FT_N)]
            for b, feed in ((0, s1), (1, [])):
                xss = []
                for qt in range(QT_N):
                    xs = mpool.tile([128, 512], BF16, tag=f"u{qt}", name=f"xs{b}_{qt}", bufs=1)
                    xss.append(xs)
                    emit_pv_qt(b, qt, xs)
                for qp in range(2):
                    xsTq2 = mpool.tile(
                        [128, 1024], BF16, tag=f"u{4 + 2 * qp}", name=f"xsTq{b}_{qp}", bufs=1
                    )
                    emit_out_qp(b, qp, xss[2 * qp : 2 * qp + 2], xsTq2, feed, alt=True)

    nc.compile()
    return nc


def _prep_inputs(q, k, v, str_mat, attn_mask, Wq, bq, Wk, bk, Wv, bv, Wo, bo):
    bf = ml_dtypes.bfloat16
    use_bias = bool(
        np.any(np.asarray(bq))
        or np.any(np.asarray(bk))
        or np.any(np.asarray(bv))
        or np.any(np.asarray(bo))
    )
    KQ = KV_N if use_bias else KT_N

    # 1/DH folded into Wq (and bq): DH = 64 = 2^6, exact in floating point.
    wqT = np.ascontiguousarray((Wq / np.float32(DH)).T).astype(np.float32)
    wkT = np.ascontiguousarray(Wk.T).astype(np.float32)
    woT = np.ascontiguousarray(Wo.T).astype(bf)

    # Pre-tile weights: [n*128, width] -> [128, n*width].
    def pretile(w):
        n = w.shape[0] // 128
        return np.ascontiguousarray(
            w.reshape(n, 128, w.shape[1]).transpose(1, 0, 2).reshape(128, -1)
        )

    if use_bias:
        wqT = np.concatenate([wqT, np.tile(bq[None, :] / np.float32(DH), (128, 1))], 0)
        wkT = np.concatenate([wkT, np.tile(bk[None, :], (128, 1))], 0)
    wqt = pretile(wqT.astype(bf))
    wkt = pretile(wkT.astype(bf))
    wot = pretile(woT)

    # Wv augmented+interleaved: chunk i, col h*65+d -> Wv.T[i*128+p, h*64+d];
    # col h*65+64 -> 0 for i<16; chunk 16 carries [bv | 1] (paired with the
    # all-(1/128) v chunk so sum_p (1/128)*x == x).
    wvT = Wv.T.astype(np.float32)  # [2048, 512]
    wva = np.zeros((KV_N * 128, VW), np.float32)
    for h in range(H):
        wva[:NLOC, h * 65 : h * 65 + 64] = wvT[:, h * 64 : (h + 1) * 64]
        wva[NLOC:, h * 65 : h * 65 + 64] = bv[None, h * 64 : (h + 1) * 64]
        wva[NLOC:, h * 65 + 64] = 1.0
    wvt = pretile(wva.astype(bf))

    bor = bo[None, :].astype(bf)
    ident = np.eye(128, dtype=bf)
    onesr = np.ones((1, 512), dtype=bf)

    q16 = np.asarray(q).astype(bf)
    k16 = np.asarray(k).astype(bf)
    v16 = np.asarray(v).astype(bf)
    onechunk = np.full((128, R), 1.0 / 128.0, dtype=bf)

    def pretile_T(x, aug):
        # [R, NLOC] -> [128, n*R] with [p, i*R+r] = x[r, i*128+p]
        t = np.ascontiguousarray(
            x.reshape(R, KT_N, 128).transpose(2, 1, 0).reshape(128, KT_N * R)
        )
        if aug:
            t = np.concatenate([t, onechunk], axis=1)
        return np.ascontiguousarray(t)

    strf = np.asarray(str_mat, dtype=np.float32)
    maskf = np.asarray(attn_mask) != 0
    mstr = np.where(maskf, strf, np.float32(-1e9))

    in_maps = []
    for c in range(NCORES):
        sl = slice(c * BL, (c + 1) * BL)
        # [BL, S, S] -> [128, BL*QT_N*S] with [p, (b*4+qt)*S+col].
        mstrt = np.ascontiguousarray(
            mstr[sl].reshape(BL * QT_N, 128, S).transpose(1, 0, 2).reshape(128, -1)
        )
        m = {
            "q": pretile_T(q16[sl].reshape(R, NLOC), use_bias),
            "k": pretile_T(k16[sl].reshape(R, NLOC), use_bias),
            "v": pretile_T(v16[sl].reshape(R, NLOC), True),
            "mstr": mstrt,
            "wqT": wqt,
            "wkT": wkt,
            "wvT": wvt,
            "woT": wot,
            "bor": bor,
            "ident": ident,
            "onesr": onesr,
        }
        in_maps.append(m)
    return in_maps, use_bias


def kernel(q, k, v, str_mat, attn_mask, Wq, bq, Wk, bk, Wv, bv, Wo, bo):
    in_maps, use_bias = _prep_inputs(
        q, k, v, str_mat, attn_mask, Wq, bq, Wk, bk, Wv, bv, Wo, bo
    )
    key = ("nc", use_bias)
    if key not in _CACHE:
        _CACHE[key] = _build(use_bias)
    nc = _CACHE[key]
    res = run_bass_kernel_spmd(nc, in_maps, core_ids=list(range(NCORES)))
    out = np.empty((B, S, NLOC), dtype=np.float32)
    for c in range(NCORES):
        out[c * BL : (c + 1) * BL] = (
            res.results[c]["out"].astype(np.float32).reshape(BL, S, NLOC)
        )
    return out
